# revision 39
# baseline (speedup 1.0000x reference)
"""EngagementPredictor TRN2 kernel: 3-branch MHA + masked mean-pool + MLP.

Sharding: pure data-parallel - B=8 batch elements, one per NeuronCore;
weights replicated; no collectives. Each core computes its [2]-logit row.

Structure (per core):
  The pool weights are zero at masked-off positions AND masked keys get
  probability 0, so the whole computation collapses to attention over only
  the masked-in positions. Host compacts x to those S_c columns (padded to
  a multiple of 64, typically 576 of 1024); every projection/attention
  matmul shrinks accordingly. Padding keys project to K=V=0 (no bias added
  on device), so exp(score)=exp(0)=1 at pads; the softmax denominator
  (free from the Exp instruction's accum_out) is corrected by -n_pad, and
  padded V rows contribute exactly 0 to the pooled output.

  Scores are computed in [q-part, k-free] layout per (head, q-tile):
  PSUM <- QT_tile^T @ KT, exp on the scalar engine with fused 1/sqrt(d)
  scale and per-partition denominator accum. The per-q context is never
  materialized: with c[q] = pw[q]/denom[q],
      pooled[d] = sum_k V[k,d] * gT[k],   gT[k] = sum_q c[q] exp[q,k]
  gT accumulates in PSUM via tiny N=1 matmuls with exp tiles stationary.
  The o-projection commutes with pooling and is folded into fus1 on the
  host (W_m = ow_m @ fus1_w[m-block]; ob folded into fus1_b), halving the
  matrix-vector tail.

  Emission is software-pipelined across branches: attention items of
  branch m (scores -> exp -> c -> lagged gT matmuls) interleave drained
  "jobs" carrying branch m+1's projection chunks and branch m-1's
  pooled/h1 tails, so the PE never idles on the scalar-engine exp latency
  and the weight DMAs for the next branch prefetch during attention.
  Scalar runs exp only (Q-bias evac moved to DVE tensor_scalar_add).

  All matmul operands are bf16 (weights converted on host; Q/K/V/exp
  evacuated from PSUM as bf16) - full PE rate at any moving width and half
  the HBM traffic. PSUM accumulation stays fp32.
"""
import numpy as np
import ml_dtypes

import concourse.bass as bass
import concourse.tile as tile
from concourse import mybir
from concourse.bass_utils import run_bass_kernel_spmd

F32 = mybir.dt.float32
BF16 = mybir.dt.bfloat16
AF = mybir.ActivationFunctionType
ALU = mybir.AluOpType

P = 128
H = 1024
NT = H // P          # 8
NCORES = 8
MHAS = [("beh", 8), ("tmp", 4), ("pat", 4)]
LAG = 5              # scores-ahead-of-g software pipeline depth

_CACHE = {}


def _chunks(n, w=512):
    out = []
    o = 0
    while o < n:
        c = min(w, n - o)
        out.append((o, c))
        o += c
    return out


def _tiles(n, w=P):
    out = [(o, min(w, n - o)) for o in range(0, n, w)]
    assert all(sz % 32 == 0 for _, sz in out)  # PE tile_size granularity
    return out


def _build_nc(S):
    TL = _tiles(S)            # 128-granular q/k tiles, last may be partial
    NKT = len(TL)
    CH = _chunks(S)           # <=512 column chunks (PSUM-bank sized)
    CH_H = _chunks(H)
    nc = bass.Bass()
    dram = {}

    def dp(name, shape, dt=BF16):
        dram[name] = nc.declare_dram_parameter(name, list(shape), dt,
                                               isOutput=False)

    dp("xT", (H, S))
    dp("pw", (P, NKT), F32)        # pool weight 1/cnt at real q, 0 at pads
    dp("nnp", (P, NKT), F32)       # -(S-cnt) at real q rows, 0 at pad rows
    for m, _ in MHAS:
        for wn in ("qw", "kw", "vw", "w1"):
            dp(f"{m}_{wn}", (H, H))
        dp(f"{m}_qb", (P, NT), F32)
        dp(f"{m}_vb", (P, NT), F32)
    dp("b1", (P, NT), F32)
    dp("fus2_w", (H, H // 2))
    dp("fus2_b", (P, 4), F32)
    dp("cls_w", (H // 2, 2))
    dp("cls_b", (1, 2), F32)
    out = nc.declare_dram_parameter("out", [1, 2], F32, isOutput=True)

    def r3(ap):  # [K, N] dram -> [P, K//P, N] partition-inner
        return ap[:].rearrange("(t p) n -> p t n", p=P)

    with tile.TileContext(nc) as tc, \
         nc.allow_low_precision(
             reason="bf16 storage/matmuls throughout; fp32 PSUM accumulation"):
        with tc.tile_pool(name="big", bufs=1) as big, \
             tc.tile_pool(name="qkv", bufs=2) as qkv, \
             tc.tile_pool(name="wstr", bufs=4) as wstr, \
             tc.tile_pool(name="expp", bufs=LAG + 2) as expp, \
             tc.tile_pool(name="small", bufs=1) as small, \
             tc.tile_pool(name="work", bufs=LAG + 2) as work, \
             tc.tile_pool(name="ppj", bufs=2, space="PSUM") as ppj, \
             tc.tile_pool(name="psc0", bufs=2, space="PSUM") as psc0, \
             tc.tile_pool(name="psc1", bufs=1, space="PSUM") as psc1, \
             tc.tile_pool(name="pg", bufs=2, space="PSUM") as pg, \
             tc.tile_pool(name="ptl", bufs=1, space="PSUM") as ptl:

            # ---- PE warm-up: back-to-back matmuls on a memset tile give
            # the HAM clock-gate a fully-busy window so the PE is at 2.4GHz
            # before the first weights arrive (cold start is ~8.5us of DMA)
            warm = small.tile([P, 512], BF16, tag="warm")
            nc.vector.memset(warm[:], 0.0)
            _wi = [0]

            def warmup(n):
                for _ in range(n):
                    wps = ppj.tile([P, 512], F32, tag="pj",
                                   name=f"warm{_wi[0]}")
                    nc.tensor.matmul(wps[:], lhsT=warm[:, :P], rhs=warm[:],
                                     start=True, stop=True)
                    _wi[0] += 1

            warmup(24)

            # ---- startup DMAs: first branch weights + x, finest-first ----
            xT = big.tile([P, NT, S], BF16, tag="xT")
            wq0 = wstr.tile([P, NT, H], BF16, tag="w", name="wt_beh_qw")
            for ki in range(NT):
                nc.sync.dma_start(wq0[:, ki], r3(dram["beh_qw"])[:, ki])
                nc.sync.dma_start(xT[:, ki, :CH[0][1]],
                                  r3(dram["xT"])[:, ki, :CH[0][1]])
            if len(CH) > 1:
                for ki in range(NT):
                    nc.sync.dma_start(xT[:, ki, CH[0][1]:],
                                      r3(dram["xT"])[:, ki, CH[0][1]:])
            pw = small.tile([P, NKT], F32, tag="pw")
            nc.sync.dma_start(pw[:], dram["pw"][:])
            nnp = small.tile([P, NKT], F32, tag="nnp")
            nc.sync.dma_start(nnp[:], dram["nnp"][:])
            b1t = small.tile([P, NT], F32, tag="b1t")
            nc.sync.dma_start(b1t[:], dram["b1"][:])

            # h1 accumulator starts at the (folded) fus1 bias
            h1acc = small.tile([P, NT], F32, tag="h1acc")
            nc.vector.tensor_copy(h1acc[:], b1t[:])

            biases = {}
            for mi, (m, _) in enumerate(MHAS):
                qb = small.tile([P, NT], F32, tag=f"qb{mi}", name=f"qb{mi}")
                nc.sync.dma_start(qb[:], dram[f"{m}_qb"][:])
                vb = small.tile([P, NT], F32, tag=f"vb{mi}", name=f"vb{mi}")
                nc.sync.dma_start(vb[:], dram[f"{m}_vb"][:])
                biases[mi] = (qb, vb)

            qkvs = {}

            def proj_jobs(mi, defer_v=False):
                """DMA-issue + per-output-chunk matmul jobs for branch mi's
                Q/K/V projections. Q-bias evac on DVE (scalar is reserved
                for exp). With defer_v the V compute jobs are returned
                separately (to drain during branch mi's own attention,
                keeping the PE busy there); the vw DMA stays in the main
                list so the data still prefetches a phase early."""
                m, nh = MHAS[mi]
                QT = qkv.tile([P, NT, S], BF16, tag="QT", name=f"QT{mi}")
                KT = qkv.tile([P, NT, S], BF16, tag="KT", name=f"KT{mi}")
                V = qkv.tile([P, NKT, H], BF16, tag="V", name=f"V{mi}")
                qkvs[mi] = (QT, KT, V)
                qb = biases[mi][0]
                jobs = []
                wts = {}

                def dma_w(wn):
                    # prefetched with >1 attention-item of slack; one DMA
                    # keeps sync-engine issue pressure low
                    wt = wstr.tile([P, NT, H], BF16, tag="w",
                                   name=f"wt_{m}_{wn}")
                    wts[wn] = wt
                    def j(wn=wn, wt=wt):
                        nc.sync.dma_start(wt[:], r3(dram[f"{m}_{wn}"]))
                    return j

                def qk_chunk(wn, dst, with_bias, ho, qo, qw_):
                    def j():
                        wt = wts[wn]
                        hsl = slice(ho * P, (ho + 1) * P)
                        qsl = slice(qo, qo + qw_)
                        ps = ppj.tile([P, 512], F32, tag="pj",
                                      name=f"pj_{m}_{wn}_{ho}_{qo}")
                        for ki in range(NT):
                            nc.tensor.matmul(
                                ps[:, :qw_], lhsT=wt[:, ki, hsl],
                                rhs=xT[:, ki, qsl],
                                start=(ki == 0), stop=(ki == NT - 1))
                        if with_bias:
                            nc.vector.tensor_scalar_add(
                                dst[:, ho, qsl], ps[:, :qw_],
                                qb[:, ho:ho + 1])
                        else:
                            nc.vector.tensor_copy(dst[:, ho, qsl],
                                                  ps[:, :qw_])
                    return j

                def v_chunk(st, sn, ho2, hw2):
                    def j():
                        wt = wts["vw"]
                        ssl = slice(st, st + sn)
                        hsl = slice(ho2, ho2 + hw2)
                        ps = ppj.tile([P, 512], F32, tag="pj",
                                      name=f"pjv_{m}_{st}_{ho2}")
                        for ki in range(NT):
                            nc.tensor.matmul(
                                ps[:sn, :hw2], lhsT=xT[:, ki, ssl],
                                rhs=wt[:, ki, hsl],
                                start=(ki == 0), stop=(ki == NT - 1))
                        nc.vector.tensor_copy(V[:sn, st // P, hsl],
                                              ps[:sn, :hw2])
                    return j

                if mi > 0:  # branch 0's qw DMA already issued at startup
                    jobs.append(dma_w("qw"))
                else:
                    wts["qw"] = wq0
                jobs.append(dma_w("kw"))
                jobs.append(dma_w("vw"))
                for ho in range(NT):
                    for qo, qw_ in CH:
                        jobs.append(qk_chunk("qw", QT, True, ho, qo, qw_))
                for ho in range(NT):
                    for qo, qw_ in CH:
                        jobs.append(qk_chunk("kw", KT, False, ho, qo, qw_))
                vjobs = []
                for st, sn in TL:
                    for ho2, hw2 in CH_H:
                        vjobs.append(v_chunk(st, sn, ho2, hw2))
                if defer_v:
                    return jobs, vjobs
                return jobs + vjobs

            def tail_jobs(mi, gTall):
                """pooled = V^T gT (+vb) then h1acc += W1^T pooled."""
                m, nh = MHAS[mi]
                ndt = (H // MHAS[mi][1]) // P
                V = qkvs[mi][2]
                vb = biases[mi][1]
                w1 = wstr.tile([P, NT, H], BF16, tag="w", name=f"wt_{m}_w1")
                pooledm = small.tile([P, NT], BF16, tag=f"pm{mi}",
                                     name=f"pm{mi}")

                def j_dma():
                    nc.sync.dma_start(w1[:], r3(dram[f"{m}_w1"]))

                def j_pooled():
                    pps = ptl.tile([P, NT], F32, tag="tail",
                                   name=f"pooled{mi}")
                    for gdt in range(NT):
                        h = gdt // ndt
                        dsl = slice(gdt * P, (gdt + 1) * P)
                        for kt, (ko, kn) in enumerate(TL):
                            nc.tensor.matmul(
                                pps[:, gdt:gdt + 1],
                                lhsT=V[:kn, kt, dsl],
                                rhs=gTall[:kn, h, kt:kt + 1],
                                start=(kt == 0), stop=(kt == NKT - 1))
                    nc.vector.tensor_add(out=pooledm[:], in0=pps[:],
                                         in1=vb[:])

                def j_h1():
                    h1ps = ptl.tile([P, NT], F32, tag="tail",
                                    name=f"h1ps{mi}")
                    for tg in range(NT):
                        tsl = slice(tg * P, (tg + 1) * P)
                        for ki in range(NT):
                            nc.tensor.matmul(
                                h1ps[:, tg:tg + 1],
                                lhsT=w1[:, ki, tsl],
                                rhs=pooledm[:, ki:ki + 1],
                                start=(ki == 0), stop=(ki == NT - 1))
                    nc.vector.tensor_add(out=h1acc[:], in0=h1ps[:],
                                         in1=h1acc[:])

                return [j_dma, j_pooled, j_h1]

            def attention(mi, jobs):
                """Scores/exp/c with LAG-deep gT pipeline; drains `jobs`
                between items to keep the PE fed during exp latency."""
                m, nh = MHAS[mi]
                d = H // nh
                ndt = d // P
                inv_sqrt_d = 1.0 / float(np.sqrt(d))
                QT, KT, V = qkvs[mi]
                gTall = small.tile([P, nh, NKT], BF16, tag=f"gT{mi}",
                                   name=f"gT{mi}")
                gts = {}
                pending = []

                def emit_scores(h, qt):
                    qo, qn = TL[qt]
                    qsl = slice(qo, qo + qn)
                    e2 = expp.tile([P, S], BF16, tag="e2",
                                   name=f"e2_{mi}_{h}_{qt}")
                    dp_ = work.tile([P, len(CH)], F32, tag="dp",
                                    name=f"dp_{mi}_{h}_{qt}")
                    for ci, (ko, kw_) in enumerate(CH):
                        ksl = slice(ko, ko + kw_)
                        pool = psc0 if ci == 0 else psc1
                        ps = pool.tile([P, 512], F32, tag=f"sc{ci}",
                                       name=f"sc{mi}_{h}_{qt}_{ci}")
                        for dt in range(ndt):
                            hd = h * ndt + dt
                            nc.tensor.matmul(
                                ps[:qn, :kw_],
                                lhsT=QT[:, hd, qsl],
                                rhs=KT[:, hd, ksl],
                                start=(dt == 0), stop=(dt == ndt - 1))
                        nc.scalar.activation(
                            e2[:qn, ksl], ps[:qn, :kw_], AF.Exp,
                            scale=inv_sqrt_d, accum_out=dp_[:qn, ci:ci + 1])
                    den = work.tile([P, 1], F32, tag="den",
                                    name=f"den_{mi}_{h}_{qt}")
                    if len(CH) > 1:  # den = (dp0 + dp1) + nnp, one DVE op
                        nc.vector.tensor_scalar(
                            den[:qn], dp_[:qn, 0:1], dp_[:qn, 1:2],
                            nnp[:qn, qt:qt + 1], op0=ALU.add, op1=ALU.add)
                    else:
                        nc.vector.tensor_scalar_add(den[:qn], dp_[:qn],
                                                    nnp[:qn, qt:qt + 1])
                    rec = work.tile([P, 1], F32, tag="rec",
                                    name=f"rec_{mi}_{h}_{qt}")
                    nc.vector.reciprocal(rec[:qn], den[:qn])
                    cbf = work.tile([P, 1], BF16, tag="cbf",
                                    name=f"cbf_{mi}_{h}_{qt}")
                    nc.vector.tensor_mul(out=cbf[:qn], in0=rec[:qn],
                                         in1=pw[:qn, qt:qt + 1])
                    return e2, cbf

                def emit_g(h, qt, e2, cbf):
                    qo, qn = TL[qt]
                    gt = gts[h]
                    for kt, (ko, kn) in enumerate(TL):
                        nc.tensor.matmul(
                            gt[:kn, kt:kt + 1],
                            lhsT=e2[:qn, ko:ko + kn],
                            rhs=cbf[:qn],
                            start=(qt == 0), stop=(qt == NKT - 1))
                    if qt == NKT - 1:
                        lkn = TL[-1][1]
                        if lkn < P:  # don't touch unwritten PSUM rows
                            nc.vector.tensor_copy(gTall[:, h, :NKT - 1],
                                                  gt[:, :NKT - 1])
                            nc.vector.tensor_copy(
                                gTall[:lkn, h, NKT - 1:NKT],
                                gt[:lkn, NKT - 1:NKT])
                        else:
                            nc.vector.tensor_copy(gTall[:, h, :],
                                                  gt[:, :NKT])

                items = [(h, qt) for h in range(nh) for qt in range(NKT)]
                ndrained = 0
                for i, (h, qt) in enumerate(items):
                    if qt == 0:
                        gts[h] = pg.tile([P, NKT], F32, tag="g",
                                         name=f"g{mi}_{h}")
                    pending.append((h, qt) + emit_scores(h, qt))
                    if len(pending) > LAG:
                        emit_g(*pending.pop(0))
                    # drain queued cross-branch jobs to fill PE slack
                    want = (len(jobs) * (i + 1)) // len(items)
                    while ndrained < want:
                        jobs[ndrained]()
                        ndrained += 1
                while pending:
                    emit_g(*pending.pop(0))
                while ndrained < len(jobs):
                    jobs[ndrained]()
                    ndrained += 1
                return gTall

            # MLP-tail weights: tiles up front, DMA issued as a job during
            # the last attention phase so the tail never waits
            w2 = wstr.tile([P, NT, H // 2], BF16, tag="w2", bufs=1)
            cw = small.tile([P, 4, 2], BF16, tag="cw")
            b2 = small.tile([P, 4], F32, tag="b2")
            cb = small.tile([1, 2], F32, tag="cb")

            def tailw_dma():
                nc.sync.dma_start(w2[:], r3(dram["fus2_w"]))
                nc.sync.dma_start(cw[:], r3(dram["cls_w"]))
                nc.sync.dma_start(b2[:], dram["fus2_b"][:])
                nc.sync.dma_start(cb[:], dram["cls_b"][:])

            # ---- pipeline: P(0); A(m) x {P(m+1), tails(m-1)}; tails.
            # The last branch's V compute defers into its own attention
            # phase, which otherwise has too little PE work to hold the
            # HAM clock gate at 2.4GHz. ----
            carry = []
            vcarry = []
            gtls = {}
            pj0_jobs = proj_jobs(0)
            last = len(MHAS) - 1
            for idx, j in enumerate(pj0_jobs):
                j()
                if idx < 10:
                    # the first ~10 projection chunks are DMA-paced; filler
                    # matmuls keep the HAM activity window busy so the PE
                    # clock doesn't fall back to 1.2GHz mid-stream
                    warmup(3)
            for mi in range(len(MHAS)):
                jobs = list(carry) + list(vcarry)
                vcarry = []
                if mi + 1 < len(MHAS):
                    if mi + 1 == last:
                        pj, vcarry = proj_jobs(mi + 1, defer_v=True)
                        jobs += pj
                    else:
                        jobs += proj_jobs(mi + 1)
                else:
                    jobs.append(tailw_dma)
                gtls[mi] = attention(mi, jobs)
                carry = tail_jobs(mi, gtls[mi])
            for j in carry:
                j()

            # ---------- MLP tail (relu/bias on DVE: no scalar round-trips,
            # h1acc was seeded with the bias) ----------
            h1T = small.tile([P, NT], BF16, tag="h1T")
            nc.vector.tensor_scalar_max(h1T[:], h1acc[:], 0.0)

            ph2 = ptl.tile([P, 4], F32, tag="tail", name="ph2")
            for tg in range(4):
                tsl = slice(tg * P, (tg + 1) * P)
                for ki in range(NT):
                    nc.tensor.matmul(
                        ph2[:, tg:tg + 1],
                        lhsT=w2[:, ki, tsl],
                        rhs=h1T[:, ki:ki + 1],
                        start=(ki == 0), stop=(ki == NT - 1))
            h2pre = small.tile([P, 4], F32, tag="h2pre")
            nc.vector.tensor_add(out=h2pre[:], in0=ph2[:], in1=b2[:])
            h2T = small.tile([P, 4], BF16, tag="h2T")
            nc.vector.tensor_scalar_max(h2T[:], h2pre[:], 0.0)

            plg = ptl.tile([1, 2], F32, tag="tail", name="plg")
            for ki in range(4):
                nc.tensor.matmul(plg[:],
                                 lhsT=h2T[:, ki:ki + 1],
                                 rhs=cw[:, ki],
                                 start=(ki == 0), stop=(ki == 3))
            lg = small.tile([1, 2], F32, tag="lgsb")
            nc.vector.tensor_add(out=lg[:], in0=plg[:], in1=cb[:])
            nc.sync.dma_start(out[:], lg[:])

    _split_multi_waits(nc)
    return nc


def _split_multi_waits(nc, max_on_inst=1, max_on_evsem=2):
    """This walrus build caps sync waits per instruction at 1 (2 for
    EventSemaphore); Tile attaches one wait per dependent proc. Spill excess
    waits onto pure-wait EventSemaphores inserted before, on the same engine -
    the engine blocks on each condition in sequence, so semantics match."""
    for f in nc.m.functions:
        for bb in f.blocks:
            insts = list(bb.instructions)
            new = []
            changed = False
            for ins in insts:
                si = ins.sync_info
                if si is not None:
                    waits = list(si.on_wait)
                    cap = (max_on_evsem
                           if isinstance(ins, mybir.InstEventSemaphore)
                           else max_on_inst)
                    if len(waits) > cap:
                        spill = waits[:-cap]
                        keep = waits[-cap:]
                        k = 0
                        while spill:
                            chunk = spill[:max_on_evsem]
                            spill = spill[max_on_evsem:]
                            new.append(mybir.InstEventSemaphore(
                                name=f"{ins.name}-wspill{k}",
                                engine=ins.engine, ins=[], outs=[],
                                sync_info=mybir.SyncInfo(on_wait=chunk,
                                                         on_update=[])))
                            k += 1
                        ins.sync_info = mybir.SyncInfo(
                            on_wait=keep, on_update=list(si.on_update))
                        changed = True
                new.append(ins)
            if changed:
                bb.instructions = new


def _get_nc(S):
    if S not in _CACHE:
        _CACHE[S] = _build_nc(S)
    return _CACHE[S]


def _prep(inputs):
    f32 = np.float32
    bf16 = ml_dtypes.bfloat16

    def cm(b, nt=NT):  # [nt*P] bias -> [P, nt] partition-inner
        return np.ascontiguousarray(
            np.asarray(b, f32).reshape(nt, P).T)

    mask = np.asarray(inputs["attention_mask"])
    cnts = mask.sum(axis=1)
    S = int(max(32, -(-int(cnts.max()) // 32) * 32))
    NKT = (S + P - 1) // P
    SPAD = NKT * P

    w1full = np.asarray(inputs["fus1_w"], f32)
    b1 = np.asarray(inputs["fus1_b"], f32).copy()
    shared = {}
    for mi, (m, _) in enumerate(MHAS):
        for wn in ("qw", "kw", "vw"):
            shared[f"{m}_{wn}"] = np.asarray(
                inputs[f"{m}_{wn}"], f32).astype(bf16)
        w1b = w1full[mi * H:(mi + 1) * H]
        ow = np.asarray(inputs[f"{m}_ow"], f32)
        shared[f"{m}_w1"] = (ow @ w1b).astype(bf16)
        b1 += np.asarray(inputs[f"{m}_ob"], f32) @ w1b
        shared[f"{m}_qb"] = cm(inputs[f"{m}_qb"])
        shared[f"{m}_vb"] = cm(inputs[f"{m}_vb"])
    shared["b1"] = cm(b1)
    shared["fus2_w"] = np.asarray(inputs["fus2_w"], f32).astype(bf16)
    shared["fus2_b"] = cm(inputs["fus2_b"], 4)
    shared["cls_w"] = np.asarray(inputs["cls_w"], f32).astype(bf16)
    shared["cls_b"] = np.asarray(inputs["cls_b"], f32).reshape(1, 2)

    x = np.asarray(inputs["hidden_states"], f32)
    in_maps = []
    for c in range(NCORES):
        im = dict(shared)
        sel = np.flatnonzero(mask[c])
        cnt = len(sel)
        xc = np.zeros((S, H), f32)
        xc[:cnt] = x[c][sel]
        im["xT"] = np.ascontiguousarray(xc.T).astype(bf16)
        pwv = np.zeros(SPAD, f32)
        pwv[:cnt] = 1.0 / max(cnt, 1)
        im["pw"] = np.ascontiguousarray(pwv.reshape(NKT, P).T)
        # -n_pad denominator fix on real q rows; 0 on pad rows so the
        # (unused) reciprocal there can't hit a zero denominator
        nnpv = np.zeros(SPAD, f32)
        nnpv[:cnt] = -float(S - cnt)
        im["nnp"] = np.ascontiguousarray(nnpv.reshape(NKT, P).T)
        in_maps.append(im)
    return S, in_maps


def kernel(**inputs) -> np.ndarray:
    S, in_maps = _prep(inputs)
    nc = _get_nc(S)
    res = run_bass_kernel_spmd(nc, in_maps, core_ids=list(range(NCORES)))
    return np.concatenate(
        [res.results[c]["out"] for c in range(NCORES)], axis=0
    ).astype(np.float32)


# revision 40
# speedup vs baseline: 1.0253x; 1.0253x over previous
"""EngagementPredictor TRN2 kernel: 3-branch MHA + masked mean-pool + MLP.

Sharding: pure data-parallel - B=8 batch elements, one per NeuronCore;
weights replicated; no collectives. Each core computes its [2]-logit row.

Structure (per core):
  The pool weights are zero at masked-off positions AND masked keys get
  probability 0, so the whole computation collapses to attention over only
  the masked-in positions. Host compacts x to those S_c columns (padded to
  a multiple of 64, typically 576 of 1024); every projection/attention
  matmul shrinks accordingly. Padding keys project to K=V=0 (no bias added
  on device), so exp(score)=exp(0)=1 at pads; the softmax denominator
  (free from the Exp instruction's accum_out) is corrected by -n_pad, and
  padded V rows contribute exactly 0 to the pooled output.

  Scores are computed in [q-part, k-free] layout per (head, q-tile):
  PSUM <- QT_tile^T @ KT, exp on the scalar engine with fused 1/sqrt(d)
  scale and per-partition denominator accum. The per-q context is never
  materialized: with c[q] = pw[q]/denom[q],
      pooled[d] = sum_k V[k,d] * gT[k],   gT[k] = sum_q c[q] exp[q,k]
  gT accumulates in PSUM via tiny N=1 matmuls with exp tiles stationary.
  The o-projection commutes with pooling and is folded into fus1 on the
  host (W_m = ow_m @ fus1_w[m-block]; ob folded into fus1_b), halving the
  matrix-vector tail.

  Emission is software-pipelined across branches: attention items of
  branch m (scores -> exp -> c -> lagged gT matmuls) interleave drained
  "jobs" carrying branch m+1's projection chunks and branch m-1's
  pooled/h1 tails, so the PE never idles on the scalar-engine exp latency
  and the weight DMAs for the next branch prefetch during attention.
  Scalar runs exp only (Q-bias evac moved to DVE tensor_scalar_add).

  All matmul operands are bf16 (weights converted on host; Q/K/V/exp
  evacuated from PSUM as bf16) - full PE rate at any moving width and half
  the HBM traffic. PSUM accumulation stays fp32.
"""
import numpy as np
import ml_dtypes

import concourse.bass as bass
import concourse.tile as tile
from concourse import mybir
from concourse.bass_utils import run_bass_kernel_spmd

F32 = mybir.dt.float32
BF16 = mybir.dt.bfloat16
AF = mybir.ActivationFunctionType
ALU = mybir.AluOpType

P = 128
H = 1024
NT = H // P          # 8
NCORES = 8
MHAS = [("beh", 8), ("tmp", 4), ("pat", 4)]
LAG = 5              # scores-ahead-of-g software pipeline depth

_CACHE = {}


def _chunks(n, w=512):
    out = []
    o = 0
    while o < n:
        c = min(w, n - o)
        out.append((o, c))
        o += c
    return out


def _tiles(n, w=P):
    out = [(o, min(w, n - o)) for o in range(0, n, w)]
    assert all(sz % 32 == 0 for _, sz in out)  # PE tile_size granularity
    return out


def _build_nc(S):
    TL = _tiles(S)            # 128-granular q/k tiles, last may be partial
    NKT = len(TL)
    CH = _chunks(S)           # <=512 column chunks (PSUM-bank sized)
    CH_H = _chunks(H)
    nc = bass.Bass()
    dram = {}

    def dp(name, shape, dt=BF16):
        dram[name] = nc.declare_dram_parameter(name, list(shape), dt,
                                               isOutput=False)

    dp("xT", (H, S))
    dp("pw", (P, NKT), F32)        # pool weight 1/cnt at real q, 0 at pads
    dp("nnp", (P, NKT), F32)       # -(S-cnt) at real q rows, 0 at pad rows
    for m, _ in MHAS:
        for wn in ("qw", "kw", "vw", "w1"):
            dp(f"{m}_{wn}", (H, H))
        dp(f"{m}_qb", (P, NT), F32)
        dp(f"{m}_vb", (P, NT), F32)
    dp("b1", (P, NT), F32)
    dp("fus2_w", (H, H // 2))
    dp("fus2_b", (P, 4), F32)
    dp("cls_w", (H // 2, 2))
    dp("cls_b", (1, 2), F32)
    out = nc.declare_dram_parameter("out", [1, 2], F32, isOutput=True)

    def r3(ap):  # [K, N] dram -> [P, K//P, N] partition-inner
        return ap[:].rearrange("(t p) n -> p t n", p=P)

    with tile.TileContext(nc) as tc, \
         nc.allow_low_precision(
             reason="bf16 storage/matmuls throughout; fp32 PSUM accumulation"):
        with tc.tile_pool(name="big", bufs=1) as big, \
             tc.tile_pool(name="qkv", bufs=2) as qkv, \
             tc.tile_pool(name="wstr", bufs=4) as wstr, \
             tc.tile_pool(name="expp", bufs=LAG + 2) as expp, \
             tc.tile_pool(name="small", bufs=1) as small, \
             tc.tile_pool(name="work", bufs=LAG + 2) as work, \
             tc.tile_pool(name="ppj", bufs=2, space="PSUM") as ppj, \
             tc.tile_pool(name="psc0", bufs=2, space="PSUM") as psc0, \
             tc.tile_pool(name="psc1", bufs=1, space="PSUM") as psc1, \
             tc.tile_pool(name="pg", bufs=2, space="PSUM") as pg, \
             tc.tile_pool(name="ptl", bufs=1, space="PSUM") as ptl:

            # ---- PE warm-up: back-to-back matmuls on a memset tile give
            # the HAM clock-gate a fully-busy window so the PE is at 2.4GHz
            # before the first weights arrive (cold start is ~8.5us of DMA)
            warm = small.tile([P, 512], BF16, tag="warm")
            nc.vector.memset(warm[:], 0.0)
            for wi in range(24):
                wps = ppj.tile([P, 512], F32, tag="pj", name=f"warm{wi}")
                nc.tensor.matmul(wps[:], lhsT=warm[:, :P], rhs=warm[:],
                                 start=True, stop=True)

            # ---- startup DMAs: first branch weights + x, finest-first ----
            xT = big.tile([P, NT, S], BF16, tag="xT")
            wq0 = wstr.tile([P, NT, H], BF16, tag="w", name="wt_beh_qw")
            for ki in range(NT):
                nc.sync.dma_start(wq0[:, ki], r3(dram["beh_qw"])[:, ki])
                nc.sync.dma_start(xT[:, ki, :CH[0][1]],
                                  r3(dram["xT"])[:, ki, :CH[0][1]])
            if len(CH) > 1:
                for ki in range(NT):
                    nc.sync.dma_start(xT[:, ki, CH[0][1]:],
                                      r3(dram["xT"])[:, ki, CH[0][1]:])
            pw = small.tile([P, NKT], F32, tag="pw")
            nc.sync.dma_start(pw[:], dram["pw"][:])
            nnp = small.tile([P, NKT], F32, tag="nnp")
            nc.sync.dma_start(nnp[:], dram["nnp"][:])
            b1t = small.tile([P, NT], F32, tag="b1t")
            nc.sync.dma_start(b1t[:], dram["b1"][:])

            # h1 accumulator starts at the (folded) fus1 bias
            h1acc = small.tile([P, NT], F32, tag="h1acc")
            nc.vector.tensor_copy(h1acc[:], b1t[:])

            biases = {}
            for mi, (m, _) in enumerate(MHAS):
                qb = small.tile([P, NT], F32, tag=f"qb{mi}", name=f"qb{mi}")
                nc.sync.dma_start(qb[:], dram[f"{m}_qb"][:])
                vb = small.tile([P, NT], F32, tag=f"vb{mi}", name=f"vb{mi}")
                nc.sync.dma_start(vb[:], dram[f"{m}_vb"][:])
                biases[mi] = (qb, vb)

            qkvs = {}

            def proj_jobs(mi, defer_v=False):
                """DMA-issue + per-output-chunk matmul jobs for branch mi's
                Q/K/V projections. Q-bias evac on DVE (scalar is reserved
                for exp). With defer_v the V compute jobs are returned
                separately (to drain during branch mi's own attention,
                keeping the PE busy there); the vw DMA stays in the main
                list so the data still prefetches a phase early."""
                m, nh = MHAS[mi]
                QT = qkv.tile([P, NT, S], BF16, tag="QT", name=f"QT{mi}")
                KT = qkv.tile([P, NT, S], BF16, tag="KT", name=f"KT{mi}")
                V = qkv.tile([P, NKT, H], BF16, tag="V", name=f"V{mi}")
                qkvs[mi] = (QT, KT, V)
                qb = biases[mi][0]
                jobs = []
                wts = {}

                def dma_w(wn):
                    # prefetched with >1 attention-item of slack; one DMA
                    # keeps sync-engine issue pressure low
                    wt = wstr.tile([P, NT, H], BF16, tag="w",
                                   name=f"wt_{m}_{wn}")
                    wts[wn] = wt
                    def j(wn=wn, wt=wt):
                        nc.sync.dma_start(wt[:], r3(dram[f"{m}_{wn}"]))
                    return j

                def qk_chunk(wn, dst, with_bias, ho, qo, qw_):
                    def j():
                        wt = wts[wn]
                        hsl = slice(ho * P, (ho + 1) * P)
                        qsl = slice(qo, qo + qw_)
                        ps = ppj.tile([P, 512], F32, tag="pj",
                                      name=f"pj_{m}_{wn}_{ho}_{qo}")
                        for ki in range(NT):
                            nc.tensor.matmul(
                                ps[:, :qw_], lhsT=wt[:, ki, hsl],
                                rhs=xT[:, ki, qsl],
                                start=(ki == 0), stop=(ki == NT - 1))
                        if with_bias:
                            nc.vector.tensor_scalar_add(
                                dst[:, ho, qsl], ps[:, :qw_],
                                qb[:, ho:ho + 1])
                        else:
                            nc.vector.tensor_copy(dst[:, ho, qsl],
                                                  ps[:, :qw_])
                    return j

                def v_chunk(st, sn, ho2, hw2):
                    def j():
                        wt = wts["vw"]
                        ssl = slice(st, st + sn)
                        hsl = slice(ho2, ho2 + hw2)
                        ps = ppj.tile([P, 512], F32, tag="pj",
                                      name=f"pjv_{m}_{st}_{ho2}")
                        for ki in range(NT):
                            nc.tensor.matmul(
                                ps[:sn, :hw2], lhsT=xT[:, ki, ssl],
                                rhs=wt[:, ki, hsl],
                                start=(ki == 0), stop=(ki == NT - 1))
                        nc.vector.tensor_copy(V[:sn, st // P, hsl],
                                              ps[:sn, :hw2])
                    return j

                if mi > 0:  # branch 0's qw DMA already issued at startup
                    jobs.append(dma_w("qw"))
                else:
                    wts["qw"] = wq0
                jobs.append(dma_w("kw"))
                jobs.append(dma_w("vw"))
                for ho in range(NT):
                    for qo, qw_ in CH:
                        jobs.append(qk_chunk("qw", QT, True, ho, qo, qw_))
                for ho in range(NT):
                    for qo, qw_ in CH:
                        jobs.append(qk_chunk("kw", KT, False, ho, qo, qw_))
                vjobs = []
                for st, sn in TL:
                    for ho2, hw2 in CH_H:
                        vjobs.append(v_chunk(st, sn, ho2, hw2))
                if defer_v:
                    return jobs, vjobs
                return jobs + vjobs

            def tail_jobs(mi, gTall):
                """pooled = V^T gT (+vb) then h1acc += W1^T pooled."""
                m, nh = MHAS[mi]
                ndt = (H // MHAS[mi][1]) // P
                V = qkvs[mi][2]
                vb = biases[mi][1]
                w1 = wstr.tile([P, NT, H], BF16, tag="w", name=f"wt_{m}_w1")
                pooledm = small.tile([P, NT], BF16, tag=f"pm{mi}",
                                     name=f"pm{mi}")

                def j_dma():
                    nc.sync.dma_start(w1[:], r3(dram[f"{m}_w1"]))

                def j_pooled():
                    pps = ptl.tile([P, NT], F32, tag="tail",
                                   name=f"pooled{mi}")
                    for gdt in range(NT):
                        h = gdt // ndt
                        dsl = slice(gdt * P, (gdt + 1) * P)
                        for kt, (ko, kn) in enumerate(TL):
                            nc.tensor.matmul(
                                pps[:, gdt:gdt + 1],
                                lhsT=V[:kn, kt, dsl],
                                rhs=gTall[:kn, h, kt:kt + 1],
                                start=(kt == 0), stop=(kt == NKT - 1))
                    nc.vector.tensor_add(out=pooledm[:], in0=pps[:],
                                         in1=vb[:])

                def j_h1():
                    h1ps = ptl.tile([P, NT], F32, tag="tail",
                                    name=f"h1ps{mi}")
                    for tg in range(NT):
                        tsl = slice(tg * P, (tg + 1) * P)
                        for ki in range(NT):
                            nc.tensor.matmul(
                                h1ps[:, tg:tg + 1],
                                lhsT=w1[:, ki, tsl],
                                rhs=pooledm[:, ki:ki + 1],
                                start=(ki == 0), stop=(ki == NT - 1))
                    nc.vector.tensor_add(out=h1acc[:], in0=h1ps[:],
                                         in1=h1acc[:])

                return [j_dma, j_pooled, j_h1]

            def attention(mi, jobs):
                """Scores/exp/c with LAG-deep gT pipeline; drains `jobs`
                between items to keep the PE fed during exp latency."""
                m, nh = MHAS[mi]
                d = H // nh
                ndt = d // P
                inv_sqrt_d = 1.0 / float(np.sqrt(d))
                QT, KT, V = qkvs[mi]
                gTall = small.tile([P, nh, NKT], BF16, tag=f"gT{mi}",
                                   name=f"gT{mi}")
                gts = {}
                pending = []

                def emit_scores(h, qt):
                    qo, qn = TL[qt]
                    qsl = slice(qo, qo + qn)
                    e2 = expp.tile([P, S], BF16, tag="e2",
                                   name=f"e2_{mi}_{h}_{qt}")
                    dp_ = work.tile([P, len(CH)], F32, tag="dp",
                                    name=f"dp_{mi}_{h}_{qt}")
                    for ci, (ko, kw_) in enumerate(CH):
                        ksl = slice(ko, ko + kw_)
                        pool = psc0 if ci == 0 else psc1
                        ps = pool.tile([P, 512], F32, tag=f"sc{ci}",
                                       name=f"sc{mi}_{h}_{qt}_{ci}")
                        for dt in range(ndt):
                            hd = h * ndt + dt
                            nc.tensor.matmul(
                                ps[:qn, :kw_],
                                lhsT=QT[:, hd, qsl],
                                rhs=KT[:, hd, ksl],
                                start=(dt == 0), stop=(dt == ndt - 1))
                        nc.scalar.activation(
                            e2[:qn, ksl], ps[:qn, :kw_], AF.Exp,
                            scale=inv_sqrt_d, accum_out=dp_[:qn, ci:ci + 1])
                    den = work.tile([P, 1], F32, tag="den",
                                    name=f"den_{mi}_{h}_{qt}")
                    if len(CH) > 1:  # den = (dp0 + dp1) + nnp, one DVE op
                        nc.vector.tensor_scalar(
                            den[:qn], dp_[:qn, 0:1], dp_[:qn, 1:2],
                            nnp[:qn, qt:qt + 1], op0=ALU.add, op1=ALU.add)
                    else:
                        nc.vector.tensor_scalar_add(den[:qn], dp_[:qn],
                                                    nnp[:qn, qt:qt + 1])
                    rec = work.tile([P, 1], F32, tag="rec",
                                    name=f"rec_{mi}_{h}_{qt}")
                    nc.vector.reciprocal(rec[:qn], den[:qn])
                    cbf = work.tile([P, 1], BF16, tag="cbf",
                                    name=f"cbf_{mi}_{h}_{qt}")
                    nc.vector.tensor_mul(out=cbf[:qn], in0=rec[:qn],
                                         in1=pw[:qn, qt:qt + 1])
                    return e2, cbf

                def emit_g(h, qt, e2, cbf):
                    qo, qn = TL[qt]
                    gt = gts[h]
                    for kt, (ko, kn) in enumerate(TL):
                        nc.tensor.matmul(
                            gt[:kn, kt:kt + 1],
                            lhsT=e2[:qn, ko:ko + kn],
                            rhs=cbf[:qn],
                            start=(qt == 0), stop=(qt == NKT - 1))
                    if qt == NKT - 1:
                        lkn = TL[-1][1]
                        if lkn < P:  # don't touch unwritten PSUM rows
                            nc.vector.tensor_copy(gTall[:, h, :NKT - 1],
                                                  gt[:, :NKT - 1])
                            nc.vector.tensor_copy(
                                gTall[:lkn, h, NKT - 1:NKT],
                                gt[:lkn, NKT - 1:NKT])
                        else:
                            nc.vector.tensor_copy(gTall[:, h, :],
                                                  gt[:, :NKT])

                items = [(h, qt) for h in range(nh) for qt in range(NKT)]
                ndrained = 0
                for i, (h, qt) in enumerate(items):
                    if qt == 0:
                        gts[h] = pg.tile([P, NKT], F32, tag="g",
                                         name=f"g{mi}_{h}")
                    pending.append((h, qt) + emit_scores(h, qt))
                    if len(pending) > LAG:
                        emit_g(*pending.pop(0))
                    # drain queued cross-branch jobs to fill PE slack
                    want = (len(jobs) * (i + 1)) // len(items)
                    while ndrained < want:
                        jobs[ndrained]()
                        ndrained += 1
                while pending:
                    emit_g(*pending.pop(0))
                while ndrained < len(jobs):
                    jobs[ndrained]()
                    ndrained += 1
                return gTall

            # MLP-tail weights: tiles up front, DMA issued as a job during
            # the last attention phase so the tail never waits
            w2 = wstr.tile([P, NT, H // 2], BF16, tag="w2", bufs=1)
            cw = small.tile([P, 4, 2], BF16, tag="cw")
            b2 = small.tile([P, 4], F32, tag="b2")
            cb = small.tile([1, 2], F32, tag="cb")

            def tailw_dma():
                nc.sync.dma_start(w2[:], r3(dram["fus2_w"]))
                nc.sync.dma_start(cw[:], r3(dram["cls_w"]))
                nc.sync.dma_start(b2[:], dram["fus2_b"][:])
                nc.sync.dma_start(cb[:], dram["cls_b"][:])

            # ---- pipeline: P(0); A(m) x {P(m+1), tails(m-1)}; tails.
            # The last branch's V compute defers into its own attention
            # phase, which otherwise has too little PE work to hold the
            # HAM clock gate at 2.4GHz. ----
            carry = []
            vcarry = []
            gtls = {}
            for j in proj_jobs(0):
                j()
            last = len(MHAS) - 1
            for mi in range(len(MHAS)):
                jobs = list(carry) + list(vcarry)
                vcarry = []
                if mi + 1 < len(MHAS):
                    if mi + 1 == last:
                        pj, vcarry = proj_jobs(mi + 1, defer_v=True)
                        jobs += pj
                    else:
                        jobs += proj_jobs(mi + 1)
                else:
                    jobs.append(tailw_dma)
                gtls[mi] = attention(mi, jobs)
                carry = tail_jobs(mi, gtls[mi])
            for j in carry:
                j()

            # ---------- MLP tail (relu/bias on DVE: no scalar round-trips,
            # h1acc was seeded with the bias) ----------
            h1T = small.tile([P, NT], BF16, tag="h1T")
            nc.vector.tensor_scalar_max(h1T[:], h1acc[:], 0.0)

            ph2 = ptl.tile([P, 4], F32, tag="tail", name="ph2")
            for tg in range(4):
                tsl = slice(tg * P, (tg + 1) * P)
                for ki in range(NT):
                    nc.tensor.matmul(
                        ph2[:, tg:tg + 1],
                        lhsT=w2[:, ki, tsl],
                        rhs=h1T[:, ki:ki + 1],
                        start=(ki == 0), stop=(ki == NT - 1))
            h2pre = small.tile([P, 4], F32, tag="h2pre")
            nc.vector.tensor_add(out=h2pre[:], in0=ph2[:], in1=b2[:])
            h2T = small.tile([P, 4], BF16, tag="h2T")
            nc.vector.tensor_scalar_max(h2T[:], h2pre[:], 0.0)

            plg = ptl.tile([1, 2], F32, tag="tail", name="plg")
            for ki in range(4):
                nc.tensor.matmul(plg[:],
                                 lhsT=h2T[:, ki:ki + 1],
                                 rhs=cw[:, ki],
                                 start=(ki == 0), stop=(ki == 3))
            lg = small.tile([1, 2], F32, tag="lgsb")
            nc.vector.tensor_add(out=lg[:], in0=plg[:], in1=cb[:])
            nc.sync.dma_start(out[:], lg[:])

    _split_multi_waits(nc)
    return nc


def _split_multi_waits(nc, max_on_inst=1, max_on_evsem=2):
    """This walrus build caps sync waits per instruction at 1 (2 for
    EventSemaphore); Tile attaches one wait per dependent proc. Spill excess
    waits onto pure-wait EventSemaphores inserted before, on the same engine -
    the engine blocks on each condition in sequence, so semantics match."""
    for f in nc.m.functions:
        for bb in f.blocks:
            insts = list(bb.instructions)
            new = []
            changed = False
            for ins in insts:
                si = ins.sync_info
                if si is not None:
                    waits = list(si.on_wait)
                    cap = (max_on_evsem
                           if isinstance(ins, mybir.InstEventSemaphore)
                           else max_on_inst)
                    if len(waits) > cap:
                        spill = waits[:-cap]
                        keep = waits[-cap:]
                        k = 0
                        while spill:
                            chunk = spill[:max_on_evsem]
                            spill = spill[max_on_evsem:]
                            new.append(mybir.InstEventSemaphore(
                                name=f"{ins.name}-wspill{k}",
                                engine=ins.engine, ins=[], outs=[],
                                sync_info=mybir.SyncInfo(on_wait=chunk,
                                                         on_update=[])))
                            k += 1
                        ins.sync_info = mybir.SyncInfo(
                            on_wait=keep, on_update=list(si.on_update))
                        changed = True
                new.append(ins)
            if changed:
                bb.instructions = new


def _get_nc(S):
    if S not in _CACHE:
        _CACHE[S] = _build_nc(S)
    return _CACHE[S]


def _prep(inputs):
    f32 = np.float32
    bf16 = ml_dtypes.bfloat16

    def cm(b, nt=NT):  # [nt*P] bias -> [P, nt] partition-inner
        return np.ascontiguousarray(
            np.asarray(b, f32).reshape(nt, P).T)

    mask = np.asarray(inputs["attention_mask"])
    cnts = mask.sum(axis=1)
    S = int(max(32, -(-int(cnts.max()) // 32) * 32))
    NKT = (S + P - 1) // P
    SPAD = NKT * P

    w1full = np.asarray(inputs["fus1_w"], f32)
    b1 = np.asarray(inputs["fus1_b"], f32).copy()
    shared = {}
    for mi, (m, _) in enumerate(MHAS):
        for wn in ("qw", "kw", "vw"):
            shared[f"{m}_{wn}"] = np.asarray(
                inputs[f"{m}_{wn}"], f32).astype(bf16)
        w1b = w1full[mi * H:(mi + 1) * H]
        ow = np.asarray(inputs[f"{m}_ow"], f32)
        shared[f"{m}_w1"] = (ow @ w1b).astype(bf16)
        b1 += np.asarray(inputs[f"{m}_ob"], f32) @ w1b
        shared[f"{m}_qb"] = cm(inputs[f"{m}_qb"])
        shared[f"{m}_vb"] = cm(inputs[f"{m}_vb"])
    shared["b1"] = cm(b1)
    shared["fus2_w"] = np.asarray(inputs["fus2_w"], f32).astype(bf16)
    shared["fus2_b"] = cm(inputs["fus2_b"], 4)
    shared["cls_w"] = np.asarray(inputs["cls_w"], f32).astype(bf16)
    shared["cls_b"] = np.asarray(inputs["cls_b"], f32).reshape(1, 2)

    x = np.asarray(inputs["hidden_states"], f32)
    in_maps = []
    for c in range(NCORES):
        im = dict(shared)
        sel = np.flatnonzero(mask[c])
        cnt = len(sel)
        xc = np.zeros((S, H), f32)
        xc[:cnt] = x[c][sel]
        im["xT"] = np.ascontiguousarray(xc.T).astype(bf16)
        pwv = np.zeros(SPAD, f32)
        pwv[:cnt] = 1.0 / max(cnt, 1)
        im["pw"] = np.ascontiguousarray(pwv.reshape(NKT, P).T)
        # -n_pad denominator fix on real q rows; 0 on pad rows so the
        # (unused) reciprocal there can't hit a zero denominator
        nnpv = np.zeros(SPAD, f32)
        nnpv[:cnt] = -float(S - cnt)
        im["nnp"] = np.ascontiguousarray(nnpv.reshape(NKT, P).T)
        in_maps.append(im)
    return S, in_maps


def kernel(**inputs) -> np.ndarray:
    S, in_maps = _prep(inputs)
    nc = _get_nc(S)
    res = run_bass_kernel_spmd(nc, in_maps, core_ids=list(range(NCORES)))
    return np.concatenate(
        [res.results[c]["out"] for c in range(NCORES)], axis=0
    ).astype(np.float32)
